# revision 41
# baseline (speedup 1.0000x reference)
"""Canny-edge BCE loss kernel for Trainium2 (8 NeuronCores, batch-parallel).

Math notes (exactness argued + verified vs the jax reference on CPU):
  * The reference binarizes to {0,255}; every Sobel magnitude is then a
    multiple of 255, so weak==strong edges and the 16-step hysteresis is an
    exact no-op.  Canny reduces to: binarize -> 3x3 Sobel -> L1 magnitude ->
    directional NMS.  Working in {0,1} scale is exact (all comparisons are
    scale invariant).
  * BCE on {0,1} edge maps takes only two values: 0 and
    C = -clip(log(max(0,1e-38)), -100).  Under XLA-CPU the fp32 denormal
    1e-38 flushes to zero, log(0)=-inf, so C == 100.0 exactly.  Hence
    loss = C * count(pred_edges != label_edges) / N.
  * NMS keep = (mag > n1) & (mag >= n2) == (mag >= max(n1+1, n2)) for the
    integer-valued magnitudes here; (n1,n2) selected by gradient direction
    with the reference's is_h -> is_v -> diag priority.

Device layout per image pair (one pred + one label image):
  one [128, 4*1028] bf16 tile; partition p, j-slice j holds image row 4p+j;
  within a slice: [pad, 512 data, pad] pixel columns, each pixel a (pred,
  label) interleaved element pair, so a +-1 pixel column shift is a +-2
  element (4-byte-aligned) offset.  Row +-1 shifts are free-dim +-1028
  offsets for 3 of 4 slices; the boundary slice is a partition-shifted
  copy produced on the TensorEngine (eye(128,k=+-1) matmul, which also
  zeroes the image border rows for free).  Elementwise ops run on
  contiguous 1D ranges (strided DVE ops measure ~3-4x slower); garbage in
  pad columns is harmless and mag's pads are re-zeroed explicitly.

Engine balance (DVE is the bottleneck at ~0.52ns/elem 2x TT, 1.04 CP):
  * binarize runs on ScalarE as sign(x - c) in {-1,+1} (c one fp32 ulp
    below the threshold so no input hits sign(0)); the affine b=(s+1)/2
    folds into the Sobel stencils' PSUM evacuation as scale=0.5 (+bias 2
    for the smoothing stencil whose rows sum to 4), which is exact.
  * mag and mag+1 live inside extended tiles [guard|mag FW|row-shift SL]
    so each NMS candidate q_dir = max(n1+1, n2) is ONE full-width DVE op;
    the boundary row-shift slices are TensorE eye(k=+-1) matmuls
    evacuated straight into the extension (the +1 via the Copy bias).
  * The 4-way direction select stays copy_predicated x3 (1x, but every
    mask/blend alternative measures worse).  Its is_h/is_v masks cost the
    DVE nothing: TensorE computes D1 = T22*ax - ay / D2 = ay - T67*ax via
    scaled-identity matmuls and the PSUM evacuation emits the uint16 mask
    directly as relu(16*D + 16*delta) (nonzero iff D >= 0; delta sits
    inside the gap to the closest negative D on the integer grid).
  * Pair 0 is special-cased against pipeline fill: it binarizes on the
    then-idle DVE and computes interior j=1,2 row stencils there too, so
    only 4 boundary stencils sit on the first pair's TensorE chain.
  * Measured dead ends: SBUF<->SBUF partition-shifted DMA row shifts are
    ~13us/slice (127 x 2KB rows); GpSimd TT is ~4x slower than DVE and
    stalls the chain; mag = ax + ay on TensorE adds two cross-engine hops
    on the critical mag->mg1->q path and loses ~11us.
"""

import numpy as np

B, H, W = 32, 512, 512
NCORES = 8
PAIRS = B // NCORES          # image pairs per core
P = 128                      # SBUF partitions
J = H // P                   # rows per partition (4)
WP = W + 2                   # padded pixel columns per j-slice
SL = 2 * WP                  # elements per j-slice (pred/label interleaved)
FW = J * SL                  # tile free width (4112)
SLK = SL + 4                 # aux row tiles carry 2-element slack each side
G = 4                        # guard elements on extended mag tiles
XW = G + FW + SL + G         # extended tile width (mag/mg1 + shifted slice)
HSL = SL // 2                # matmul free-dim half (PSUM bank limit)
CTH = float(np.float32(0.5000001))
CLO = float(np.nextafter(np.float32(0.5000001), np.float32(0.0), dtype=np.float32))
T22 = float(np.float32(0.4142135623730951))
T67 = float(np.float32(2.414213562373095))
N_TOT = B * H * W

_CACHE = {}


def _bce_constant() -> float:
    """-clip(log(max(0,1e-38)), -100) exactly as the jax reference computes
    it on CPU (XLA flushes the fp32 denormal -> log(0) -> -inf -> clip)."""
    try:
        import jax
        import jax.numpy as jnp

        cpu = jax.devices("cpu")[0]
        with jax.default_device(cpu):
            v = jnp.clip(jnp.log(jnp.maximum(jnp.float32(0.0), 1e-38)), -100.0, None)
            return -float(v)
    except Exception:
        return 100.0


NMAT = 20


def _shift_mats() -> np.ndarray:
    """lhsT stencil matrices, [128, 10*128]: out[m] = sum_k mat[k,m] in[k].
    0 SUP (in[m+1]), 1 SDN (in[m-1]), 2 E127, 3 I, 4 I2, 5 -I, 6 -SDN,
    7 M0 = I2+E0, 8 M127 = I2+E127, 9 -E0, 10 T22*I, 11 -T67*I,
    12 -2I, 13 -M0, 14 -M127, 15 -SUP, 16 -2*SDN, 17 -2*E0,
    18 2*SUP, 19 2*E127."""
    import ml_dtypes

    eye = np.eye(P, dtype=np.float32)
    e0 = np.zeros((P, P), np.float32)
    e0[0, 0] = 1.0
    e127 = np.zeros((P, P), np.float32)
    e127[P - 1, P - 1] = 1.0
    mats = [
        np.eye(P, k=-1, dtype=np.float32),   # SUP
        np.eye(P, k=1, dtype=np.float32),    # SDN
        e127,                                # E127
        eye,                                 # I
        2.0 * eye,                           # I2
        -eye,                                # NI
        -np.eye(P, k=1, dtype=np.float32),   # NSDN
        2.0 * eye + e0,                      # M0
        2.0 * eye + e127,                    # M127
        -e0,                                 # NE0
        0.4142135623730951 * eye,            # T22I
        -2.414213562373095 * eye,            # NT67I
        -2.0 * eye,                          # N2I
        -(2.0 * eye + e0),                   # NM0
        -(2.0 * eye + e127),                 # NM127
        -np.eye(P, k=-1, dtype=np.float32),  # NSUP
        -2.0 * np.eye(P, k=1, dtype=np.float32),  # N2SDN
        -2.0 * e0,                           # N2E0
        2.0 * np.eye(P, k=-1, dtype=np.float32),  # S2UP
        2.0 * e127,                          # E2127
    ]
    return np.concatenate(mats, axis=1).astype(ml_dtypes.bfloat16)


def _build_program():
    from concourse import bacc, mybir, tile

    dt = mybir.dt
    Alu = mybir.AluOpType
    Act = mybir.ActivationFunctionType

    nc = bacc.Bacc(
        "TRN2",
        target_bir_lowering=False,
        debug=False,
        enable_asserts=False,
        num_devices=NCORES,
    )
    pred = nc.dram_tensor("pred", [PAIRS, H, W], dt.float32, kind="ExternalInput").ap()
    labels = nc.dram_tensor(
        "labels", [PAIRS, H, W], dt.float32, kind="ExternalInput"
    ).ap()
    shifts = nc.dram_tensor(
        "shifts", [P, NMAT * P], dt.bfloat16, kind="ExternalInput"
    ).ap()
    accd = nc.dram_tensor(
        "acc_out", [P, PAIRS], dt.float32, kind="ExternalOutput"
    ).ap()

    # HBM view: partition p <- rows 4p..4p+3 (contiguous 8KB per partition)
    pred_r = pred.rearrange("b (p j) w -> b p (j w)", j=J)
    labels_r = labels.rearrange("b (p j) w -> b p (j w)", j=J)

    def v2(t):  # [P, J, SL] j-slice view
        return t[:].rearrange("p (j c) -> p j c", j=J)

    def v4(t):  # [P, J, WP, 2] pixel/lane view
        return t[:].rearrange("p (j c e) -> p j c e", j=J, e=2)

    with tile.TileContext(nc) as tc:
        with (
            tc.tile_pool(name="xs", bufs=2) as xpool,
            tc.tile_pool(name="bb", bufs=2) as bpool,
            tc.tile_pool(name="mid", bufs=1) as mid,
            tc.tile_pool(name="mid2", bufs=2) as mid2,
            tc.tile_pool(name="aux", bufs=1) as aux,
            tc.tile_pool(name="cst", bufs=1) as cpool,
            tc.tile_pool(name="ps", bufs=8, space="PSUM") as psum,
            tc.tile_pool(name="accp", bufs=1) as accp,
        ):
            acc = accp.tile([P, PAIRS], dt.float32, tag="acc")
            shm = cpool.tile([P, NMAT * P], dt.bfloat16, tag="shm")
            nc.sync.dma_start(shm[:], shifts[:])
            # per-partition bias AP for the Sign binarize (activation wants
            # non-Copy biases as APs; only 0/1 are pre-registered consts)
            bth = cpool.tile([P, 1], dt.float32, tag="bth")
            nc.vector.memset(bth[:], -CLO)
            br1 = cpool.tile([P, 1], dt.float32, tag="br1")
            nc.vector.memset(br1[:], 1.28)
            br2 = cpool.tile([P, 1], dt.float32, tag="br2")
            nc.vector.memset(br2[:], 3.2)
            # extended mag tiles live across all pairs (bufs=1 anyway); the
            # guard/pad regions are initialized once before the loop
            magx = aux.tile([P, XW], dt.bfloat16, tag="magx")
            mg1x = aux.tile([P, XW], dt.bfloat16, tag="mg1x")
            mat = [shm[:, i * P : (i + 1) * P] for i in range(NMAT)]
            (SUP, SDN, E127, I, I2, NI, NSDN, M0, M127, NE0, T22I, NT67I,
             N2I, NM0, NM127, NSUP, N2SDN, N2E0, S2UP, E2127) = mat

            def stencil(dst, dst_off, terms, bias=0.0, scale=1.0, act=None):
                """dst[p, dst_off+2+i] = bias + scale * sum_t mat_t.T @ src_t
                over the 1024 data elements of a slice, via accumulating
                [128,128]@[128,512] matmuls in PSUM + a ScalarE evacuation
                (Copy takes free float scale/bias).  Pad columns are NOT
                produced; callers fill them."""
                n = len(terms)
                for h in range(2):
                    ps = psum.tile([P, W], dt.float32, tag="ps")
                    for i, (m_, src, soff) in enumerate(terms):
                        lo = soff + 2 + h * W
                        nc.tensor.matmul(
                            ps[:], m_, src[:, lo : lo + W],
                            start=(i == 0), stop=(i == n - 1),
                        )
                    dlo = dst_off + 2 + h * W
                    nc.scalar.activation(
                        dst[:, dlo : dlo + W], ps[:], act or Act.Copy,
                        bias=bias, scale=scale,
                    )

            for k in range(PAIRS):
                xp = xpool.tile([P, J * W], dt.float32, tag="xp")
                xl = xpool.tile([P, J * W], dt.float32, tag="xl")
                nc.sync.dma_start(xp[:], pred_r[k])
                nc.sync.dma_start(xl[:], labels_r[k])

                b = bpool.tile([P, FW], dt.bfloat16, tag="b")
                xpv = xp[:].rearrange("p (j w e) -> p j w e", j=J, e=1)
                xlv = xl[:].rearrange("p (j w e) -> p j w e", j=J, e=1)
                # binarize: pairs 1..3 use a ScalarE sign(x - c) in {-1,+1}
                # (c one fp32 ulp below the threshold so no input hits the
                # sign(0)=0 knife-edge; the affine b=(s+1)/2 folds into the
                # linear stencils: t rows sum to 4, u rows to 0, so evac
                # scale=0.5 / t-bias=2 is exact).  Pair 0 binarizes on the
                # DVE instead: at the pipeline head the DVE is idle anyway
                # and ScalarE then starts the PSUM evacuations sooner.
                if k == 0:
                    nc.vector.tensor_scalar(
                        v4(b)[:, :, 1 : 1 + W, 0:1], xpv, CTH, None, Alu.is_ge
                    )
                    nc.vector.tensor_scalar(
                        v4(b)[:, :, 1 : 1 + W, 1:2], xlv, CTH, None, Alu.is_ge
                    )
                    tsc, tbi = 1.0, 0.0
                else:
                    nc.scalar.activation(
                        v4(b)[:, :, 1 : 1 + W, 0:1], xpv, Act.Sign, bias=bth[:]
                    )
                    nc.scalar.activation(
                        v4(b)[:, :, 1 : 1 + W, 1:2], xlv, Act.Sign, bias=bth[:]
                    )
                    tsc, tbi = 0.5, 2.0
                # (b's own pad columns are never read: the stencils consume
                # data elements only, and x-replication is applied to t/u.)

                # Sobel row stencils straight on the TensorEngine:
                #   t[p,j] = b(4p+j-1) + 2 b(4p+j) + b(4p+j+1)   (replicate)
                #   u[p,j] = b(4p+j+1) - b(4p+j-1)
                # Slice j+-1 is a free-dim offset; the boundary slice comes
                # from the partition-shift matrices, with one-hot fix-ups
                # folded into M0/M127/NE0/E127 for the image border rows.
                # For pair 0 the ~23us serial matmul+ldweights chain IS the
                # pipeline-fill critical path, so its interior slices j=1,2
                # (pure free-dim offsets) run on the then-idle DVE and only
                # the j=0/j=3 boundary stencils use the TensorEngine.
                u = mid2.tile([P, FW], dt.bfloat16, tag="u")
                gx = mid2.tile([P, FW], dt.bfloat16, tag="gx")
                stencil(u, 0, [(I, b, SL), (NSDN, b, 3 * SL), (NE0, b, 0)],
                        scale=tsc)
                stencil(u, 3 * SL, [(SUP, b, 0), (E127, b, 3 * SL), (NI, b, 2 * SL)],
                        scale=tsc)
                if k == 0:
                    t = mid2.tile([P, FW], dt.bfloat16, tag="t")
                    stencil(t, 0, [(SDN, b, 3 * SL), (M0, b, 0), (I, b, SL)],
                            bias=tbi, scale=tsc)
                    stencil(t, 3 * SL,
                            [(I, b, 2 * SL), (M127, b, 3 * SL), (SUP, b, 0)],
                            bias=tbi, scale=tsc)
                    bv = v2(b)
                    tv = v2(t)
                    uv = v2(u)
                    A = mid2.tile([P, FW], dt.bfloat16, tag="gx")
                    Av = A[:].rearrange("p (j c) -> p j c", j=J)
                    nc.vector.tensor_tensor(
                        Av[:, 1:3, 2 : SL - 2], bv[:, 0:2, 2 : SL - 2],
                        bv[:, 2:4, 2 : SL - 2], Alu.add,
                    )
                    b2 = mid2.tile([P, FW], dt.bfloat16, tag="r")
                    b2v = b2[:].rearrange("p (j c) -> p j c", j=J)
                    nc.vector.tensor_scalar(
                        b2v[:, 1:3, 2 : SL - 2], bv[:, 1:3, 2 : SL - 2],
                        2.0, None, Alu.mult,
                    )
                    nc.vector.tensor_tensor(
                        tv[:, 1:3, 2 : SL - 2], Av[:, 1:3, 2 : SL - 2],
                        b2v[:, 1:3, 2 : SL - 2], Alu.add,
                    )
                    nc.vector.tensor_tensor(
                        uv[:, 1:3, 2 : SL - 2], bv[:, 2:4, 2 : SL - 2],
                        bv[:, 0:2, 2 : SL - 2], Alu.subtract,
                    )
                else:
                    stencil(u, SL, [(I, b, 2 * SL), (NI, b, 0)], scale=tsc)
                    stencil(u, 2 * SL, [(I, b, 3 * SL), (NI, b, SL)], scale=tsc)
                    # gx directly as a 6-term TensorE stencil (x-diff of the
                    # row-smoothing): replaces the t stencils AND the DVE
                    # x-diff TT for the steady-state pairs.  The +-1-pixel
                    # column offsets read b's replicate pads.
                    nc.vector.tensor_copy(v2(b)[:, :, 0:2], v2(b)[:, :, 2:4])
                    nc.vector.tensor_copy(
                        v2(b)[:, :, SL - 2 : SL], v2(b)[:, :, SL - 4 : SL - 2]
                    )
                    # evacs write data cols only; pads feed pad outputs
                    nc.vector.memset(v2(gx)[:, :, 0:2], 0.0)
                    nc.vector.memset(v2(gx)[:, :, SL - 2 : SL], 0.0)
                    stencil(gx, 0, [(NSDN, b, 3 * SL - 2), (SDN, b, 3 * SL + 2),
                                    (NM0, b, -2), (M0, b, 2),
                                    (NI, b, SL - 2), (I, b, SL + 2)], scale=tsc)
                    stencil(gx, SL, [(NI, b, -2), (I, b, 2),
                                     (N2I, b, SL - 2), (I2, b, SL + 2),
                                     (NI, b, 2 * SL - 2), (I, b, 2 * SL + 2)],
                            scale=tsc)
                    stencil(gx, 2 * SL, [(NI, b, SL - 2), (I, b, SL + 2),
                                         (N2I, b, 2 * SL - 2), (I2, b, 2 * SL + 2),
                                         (NI, b, 3 * SL - 2), (I, b, 3 * SL + 2)],
                            scale=tsc)
                    stencil(gx, 3 * SL, [(NI, b, 2 * SL - 2), (I, b, 2 * SL + 2),
                                         (NM127, b, 3 * SL - 2), (M127, b, 3 * SL + 2),
                                         (NSUP, b, -2), (SUP, b, 2)], scale=tsc)
                # replicate pad columns (x-shift consumers read them)
                if k == 0:
                    nc.vector.tensor_copy(v2(t)[:, :, 0:2], v2(t)[:, :, 2:4])
                    nc.vector.tensor_copy(
                        v2(t)[:, :, SL - 2 : SL], v2(t)[:, :, SL - 4 : SL - 2]
                    )
                    nc.vector.tensor_tensor(
                        gx[:, 2 : FW - 2], t[:, 4:FW], t[:, 0 : FW - 4],
                        Alu.subtract,
                    )
                nc.vector.tensor_copy(v2(u)[:, :, 0:2], v2(u)[:, :, 2:4])
                nc.vector.tensor_copy(v2(u)[:, :, SL - 2 : SL], v2(u)[:, :, SL - 4 : SL - 2])
                r = mid2.tile([P, FW], dt.bfloat16, tag="r")
                nc.vector.tensor_tensor(
                    r[:, 0 : FW - 2], u[:, 0 : FW - 2], u[:, 2:FW], Alu.add
                )
                gy = mid.tile([P, FW], dt.bfloat16, tag="gy")
                nc.vector.tensor_tensor(
                    gy[:, 2 : FW - 2], r[:, 0 : FW - 4], r[:, 2 : FW - 2], Alu.add
                )

                ax = mid.tile([P, FW], dt.bfloat16, tag="ax")
                ay = mid.tile([P, FW], dt.bfloat16, tag="ay")
                nc.scalar.activation(ax[:, 2 : FW - 2], gx[:, 2 : FW - 2], Act.Abs)
                nc.scalar.activation(ay[:, 2 : FW - 2], gy[:, 2 : FW - 2], Act.Abs)

                # Extended tiles: magx = [guard G][mag FW][mag row 4p+4 SL][G],
                # mg1x = [G][mag+1 row 4p-1 SL][mag+1 FW][G].  The row-shifted
                # slices come from partition-shifted SBUF->SBUF DMAs (idle DMA
                # rings) instead of TensorE matmuls, and give every NMS
                # neighbor of every j-slice as a single contiguous view, so
                # each of the four q candidates is ONE full-width DVE op.
                mag = magx[:, G : G + FW]
                mg1 = mg1x[:, G + SL : G + SL + FW]
                nc.vector.tensor_tensor(
                    mag[:, 2 : FW - 2], ax[:, 2 : FW - 2], ay[:, 2 : FW - 2], Alu.add
                )
                # NMS uses a zero border: zero every pad column (also covers
                # the mag region's first/last two elements)
                magv = mag.rearrange("p (j c) -> p j c", j=J)
                nc.vector.memset(magv[:, :, 0:2], 0.0)
                nc.vector.memset(magv[:, :, SL - 2 : SL], 0.0)
                # n1-side comparisons need mag+1 (strict >); pads become 1,
                # which is also the correct zero-border n1+1 value.
                nc.vector.tensor_scalar(mg1, mag, 1.0, None, Alu.add)
                # row 4p+4 = partition p+1's j=0 slice of mag (n2 side, SUP
                # matmul zeroes the image border row); row 4p-1 = partition
                # p-1's j=3 slice of mag (n1 side, SDN matmul) with the
                # strict-inequality +1 folded into the evacuation bias (which
                # also turns the zero border into the correct n1+1 = 1).
                mnr = magx[:, G + FW : G + FW + SL]
                mpr = mg1x[:, G : G + SL]
                stencil(mnr, 0, [(SUP, mag, 0)])
                stencil(mpr, 0, [(SDN, mag, 3 * SL)], bias=1.0)
                # region pads + guard columns feed only pad outputs and are
                # never overwritten by the per-pair compute: set them once
                if k == 0:
                    for reg in (mnr, mpr):
                        nc.vector.memset(reg[:, 0:2], 0.0)
                        nc.vector.memset(reg[:, SL - 2 : SL], 0.0)
                    nc.vector.memset(mg1x[:, 0:G], 0.0)
                    nc.vector.memset(magx[:, G + FW + SL : XW], 0.0)

                # direction predicates (contiguous, data region only).
                gg = mid.tile([P, FW], dt.bfloat16, tag="c1")
                nc.vector.tensor_tensor(
                    gg[:, 2 : FW - 2], gx[:, 2 : FW - 2], gy[:, 2 : FW - 2], Alu.mult
                )
                dp = mid2.tile([P, FW], dt.uint16, tag="t")
                nc.vector.tensor_scalar(
                    dp[:, 2 : FW - 2], gg[:, 2 : FW - 2], 0.0, None, Alu.is_ge
                )
                # is_h / is_v masks with ZERO DVE ops: the TensorEngine
                # computes D1 = T22*ax - ay and D2 = ay - T67*ax (scaled
                # identity matrices), and the PSUM evacuation itself builds
                # the copy_predicated mask via Relu with a small bias:
                # relu(16*D + 16*d) as uint16 is nonzero iff D >= 0: on the integer
                # (ax, ay) grid the closest negative D1 is -0.17 (d=0.08) and
                # the closest negative D2 is -0.414 (d=0.2); D=0 maps to
                # 16*d >= 1.28 which survives the uint16 convert as >= 1
                # (copy_predicated requires an integer mask dtype).
                ish = mid.tile([P, FW], dt.uint16, tag="ish")
                isv = mid.tile([P, FW], dt.uint16, tag="isv")
                for msk in (ish, isv):
                    nc.vector.memset(v2(msk)[:, :, 0:2], 0.0)
                    nc.vector.memset(v2(msk)[:, :, SL - 2 : SL], 0.0)
                for j in range(J):
                    stencil(ish, j * SL, [(T22I, ax, j * SL), (NI, ay, j * SL)],
                            act=Act.Relu, bias=br1[:], scale=16.0)
                    stencil(isv, j * SL, [(I, ay, j * SL), (NT67I, ax, j * SL)],
                            act=Act.Relu, bias=br2[:], scale=16.0)

                # q_dir = max(n1+1, n2), one full-width TT per direction:
                # n1 views come from mg1x (mg1 region offset G+SL), n2 views
                # from magx (mag region offset G); row+-1 is a -+SL offset
                # covering the DMA'd boundary slices, col+-1 a +-2 offset.
                q = mid2.tile([P, FW], dt.bfloat16, tag="u")
                # q := q_d2 (n1=NE=row-1,col+1 ; n2=SW=row+1,col-1)
                nc.vector.tensor_tensor(
                    q[:],
                    mg1x[:, G + 2 : G + 2 + FW],
                    magx[:, G + SL - 2 : G + SL - 2 + FW],
                    Alu.max,
                )
                # q_d1 (n1=NW=row-1,col-1 ; n2=SE=row+1,col+1)
                qd1 = mid2.tile([P, FW], dt.bfloat16, tag="gx")
                nc.vector.tensor_tensor(
                    qd1[:],
                    mg1x[:, G - 2 : G - 2 + FW],
                    magx[:, G + SL + 2 : G + SL + 2 + FW],
                    Alu.max,
                )
                # q_v (n1=N=row-1 ; n2=S=row+1)
                qv = mid.tile([P, FW], dt.bfloat16, tag="ax")
                nc.vector.tensor_tensor(
                    qv[:],
                    mg1x[:, G : G + FW],
                    magx[:, G + SL : G + SL + FW],
                    Alu.max,
                )
                # q_h (n1=W=col-1 ; n2=E=col+1)
                qh = mid.tile([P, FW], dt.bfloat16, tag="ay")
                nc.vector.tensor_tensor(
                    qh[:],
                    mg1x[:, G + SL - 2 : G + SL - 2 + FW],
                    magx[:, G + 2 : G + 2 + FW],
                    Alu.max,
                )

                # priority select: d2 -> d1 (diag_pos) -> v (is_v) -> h (is_h)
                nc.vector.copy_predicated(
                    q[:, 2 : FW - 2], dp[:, 2 : FW - 2], qd1[:, 2 : FW - 2]
                )
                nc.vector.copy_predicated(
                    q[:, 2 : FW - 2], isv[:, 2 : FW - 2], qv[:, 2 : FW - 2]
                )
                nc.vector.copy_predicated(
                    q[:, 2 : FW - 2], ish[:, 2 : FW - 2], qh[:, 2 : FW - 2]
                )

                keep = mid2.tile([P, FW], dt.bfloat16, tag="r")
                nc.vector.tensor_tensor(
                    keep[:, 2 : FW - 2], mag[:, 2 : FW - 2], q[:, 2 : FW - 2], Alu.is_ge
                )

                # d = (keep_pred != keep_label), accumulated count per partition
                d = mid.tile([P, J * W], dt.bfloat16, tag="d")
                dv = d[:].rearrange("p (j w e) -> p j w e", j=J, e=1)
                nc.vector.scalar_tensor_tensor(
                    dv,
                    v4(keep)[:, :, 1 : 1 + W, 0:1],
                    1.0,
                    v4(keep)[:, :, 1 : 1 + W, 1:2],
                    Alu.mult,
                    Alu.not_equal,
                    accum_out=acc[:, k : k + 1],
                )

            nc.sync.dma_start(accd[:], acc[:])

    nc.compile()
    return nc


def _get_program():
    if "nc" not in _CACHE:
        _CACHE["nc"] = _build_program()
    return _CACHE["nc"]


def kernel(pred: np.ndarray, labels: np.ndarray) -> np.ndarray:
    from concourse import bass_utils

    pred = np.asarray(pred).reshape(B, H, W).astype(np.float32, copy=False)
    labels = np.asarray(labels).reshape(B, H, W).astype(np.float32, copy=False)

    nc = _get_program()
    shifts = _shift_mats()
    in_maps = []
    for c in range(NCORES):
        sl = slice(c * PAIRS, (c + 1) * PAIRS)
        in_maps.append(
            {
                "pred": np.ascontiguousarray(pred[sl]),
                "labels": np.ascontiguousarray(labels[sl]),
                "shifts": shifts,
            }
        )
    res = bass_utils.run_bass_kernel_spmd(nc, in_maps, core_ids=list(range(NCORES)))
    k_total = sum(float(r["acc_out"].astype(np.float64).sum()) for r in res.results)
    loss = np.float32(_bce_constant() * k_total / float(N_TOT))
    return np.array(loss, dtype=np.float32)

